# revision 11
# baseline (speedup 1.0000x reference)
"""Trainium2 Bass kernel for nn_MDDecoder (Poincare-ball decoder loss).

Math (c = 1):
  With nx = ||x_i||^2, px = 1 - nx, the Poincare sqdist reduces (exactly) to
      dist(i,j) = 2*asinh(sqrt(Etil)) = ln(1 + E2 + sqrt((E2+1)^2 - 1))
  where Etil = ||x_i - x_j||^2 / (px_i * px_j) and E2 = 2*Etil.
  E2 is a bilinear form: E2 = sum_k a[k,i] * b[k,j] with 18 features
      a = [x/px; nx/px; 1/px],  b = [-4x/py; 2/py; 2ny/py]
  so one K=18 matmul per tile produces E2 (run as K=54 via bf16 hi/lo
  3-term split for ~2^-18 precision at full PE rate).

  loss = sum_{adj=1} (sq(i,j) + ln(negsum_i)),
  negsum_i = sum_j exp(-sq) * (1-adj) = sumsimi_i - possimi_i.

Sharding: rows of the [N,N] matrix across 8 cores (512 rows each); x-derived
feature tables replicated. Scalar outputs combined on host from per-row
device partials.
"""
import os
import sys

sys.path.insert(0, "/opt/trn_rl_repo")

import numpy as np
import ml_dtypes

import concourse.bass as bass
import concourse.bacc as bacc
import concourse.tile as tile
from concourse import mybir
from concourse.bass_utils import run_bass_kernel_spmd

N = 4096
DIM = 16
NCORES = 8
R = N // NCORES          # rows per core = 512
RT = R // 128            # row tiles per core = 4
JC = N // 512            # psum chunks per row = 8
DELTA = 3e-5             # sqrt-input safety bias (abs error floor of E2)

F32 = mybir.dt.float32
BF16 = mybir.dt.bfloat16
I32 = mybir.dt.int32
AF = mybir.ActivationFunctionType
ALU = mybir.AluOpType

_CACHE = {}

from concourse.dve_spec import Spec as _Spec, Src0 as _S0, Src1 as _S1, \
    C1 as _C1, C2 as _C2, maxx as _maxx, lower as _dve_lower, \
    _has_src1 as _hs1
from concourse.dve_uop import DveOpSpec as _DveOpSpec
import concourse.dve_ops as _dve_ops


def _register_custom(name, spec):
    for op in _dve_ops.OPS:
        if op.name == name:
            return op
    row = _dve_ops._CUSTOM_DVE_ROW_BASE + len(_dve_ops.OPS)
    assert row < 0x20
    _dve_ops._SUB_OPCODE_FOR_NAME[name] = row
    u = _dve_lower(spec, ver="v3")
    sh = _DveOpSpec(name=name, opcode=row, uops=u, rd1_en=_hs1(spec)).sha("v3")
    op = _dve_ops.DveOp(name, spec, subdim=False, uops_sha={"v3": sh})
    _dve_ops.OPS.append(op)
    _dve_ops.CUSTOM_DVE_SPECS[name] = spec
    return op


def _ref_add_max(in0, in1, c0, c1, c2):
    b = (in0.astype(np.float32) + in1).astype(np.float32)
    m = np.maximum(c1, b.reshape(b.shape[0], -1).max(axis=-1, keepdims=True))
    return b, m.astype(np.float32)


def _ref_mul_max(in0, in1, c0, c1, c2):
    b = (in0.astype(np.float32) * in1 * c2).astype(np.float32)
    m = np.maximum(c1, b.reshape(b.shape[0], -1).max(axis=-1, keepdims=True))
    return b, m.astype(np.float32)


ADD_MAXACC = _register_custom(
    "ADD_MAXACC",
    _Spec(body=_S0 + _S1, accum=_maxx, accum_init=_C1, reference=_ref_add_max))
MUL_MAXACC = _register_custom(
    "MUL_MAXACC",
    _Spec(body=_S0 * _S1 * _C2, accum=_maxx, accum_init=_C1,
          reference=_ref_mul_max))



def _bf16_split3(v):
    """3-term bf16 split of matmul feature tables.

    v [K, M] float64 -> stationary/moving row triples so that
    sum(hi*hi' + hi*lo' + lo*hi') ~= v*v' with ~2^-18 relative error.
    Returns (first, second) row blocks to concat: caller pairs
    stationary [ah; ah; al] with moving [bh; bl; bh].
    """
    hi = v.astype(ml_dtypes.bfloat16)
    lo = (v - hi.astype(np.float64)).astype(ml_dtypes.bfloat16)
    return hi, lo


def build_program(n_rows=R, width=N):
    """Build the SPMD bass program for one core owning n_rows rows."""
    rt = n_rows // 128
    jc = width // 512
    KE = 54   # E2 features after bf16x3 split (3*18)
    KG = 48   # G  features after bf16x3 split (3*16)

    nc = bacc.Bacc("TRN2", target_bir_lowering=False, debug=False, num_devices=1)

    st_e = nc.dram_tensor("st_e", [KE, n_rows], BF16, kind="ExternalInput").ap()
    mv_e = nc.dram_tensor("mv_e", [KE, width], BF16, kind="ExternalInput").ap()
    st_g = nc.dram_tensor("st_g", [KG, n_rows], BF16, kind="ExternalInput").ap()
    mv_g = nc.dram_tensor("mv_g", [KG, width], BF16, kind="ExternalInput").ap()
    adj_b = nc.dram_tensor("adj_b", [n_rows, width], I32, kind="ExternalInput").ap()

    sq_out = nc.dram_tensor("sq_out", [n_rows, width], F32, kind="ExternalOutput").ap()
    loss_out = nc.dram_tensor("loss_out", [rt, 128], F32, kind="ExternalOutput").ap()
    maxsq_out = nc.dram_tensor("maxsq_out", [rt, 128], F32, kind="ExternalOutput").ap()
    maxge_out = nc.dram_tensor("maxge_out", [rt, 128], F32, kind="ExternalOutput").ap()
    minge_out = nc.dram_tensor("minge_out", [rt, 128], F32, kind="ExternalOutput").ap()
    negsum_out = nc.dram_tensor("negsum_out", [rt, 128], F32, kind="ExternalOutput").ap()

    with tile.TileContext(nc) as tc:
        with tc.tile_pool(name="const", bufs=1) as cpool, \
             tc.tile_pool(name="chunk", bufs=2) as kpool, \
             tc.tile_pool(name="u3", bufs=3) as upool, \
             tc.tile_pool(name="w1", bufs=1) as w1pool, \
             tc.tile_pool(name="adj2", bufs=2) as apool, \
             tc.tile_pool(name="b16", bufs=3) as bpool, \
             tc.tile_pool(name="vecs", bufs=rt) as vpool, \
             tc.tile_pool(name="psum", bufs=2, space="PSUM") as psum:

            st_e_sb = cpool.tile([KE, n_rows], BF16, tag="st_e")
            nc.sync.dma_start(st_e_sb[:], st_e[:])
            mv_e_sb = cpool.tile([KE, width], BF16, tag="mv_e")
            nc.sync.dma_start(mv_e_sb[:], mv_e[:])
            st_g_sb = cpool.tile([KG, n_rows], BF16, tag="st_g")
            nc.sync.dma_start(st_g_sb[:], st_g[:])
            mv_g_sb = cpool.tile([KG, width], BF16, tag="mv_g")
            nc.sync.dma_start(mv_g_sb[:], mv_g[:])

            bias_s1 = cpool.tile([128, 1], F32, tag="bias_s1")
            nc.gpsimd.memset(bias_s1[:], DELTA - 1.0)

            # alpha/beta phases batched over PAIRS of row tiles so the ACT
            # table set switches (sqrt <-> ln/exp) happen per pair, not per op.
            n1024 = width // 1024
            vec = {}
            acts = {"first_alpha": None, "beta_list": []}

            def alpha(it, u, adj_t):
                r0 = it * 128
                maxu_acc = vpool.tile([128, n1024], F32, tag="maxu_acc")
                maxge_acc = vpool.tile([128, n1024], F32, tag="maxge_acc")
                negge_acc = vpool.tile([128, n1024], F32, tag="negge_acc")
                for j in range(n1024):
                    e2_ps = psum.tile([128, 1024], F32, tag="e2")
                    for h in range(2):
                        lo = j * 1024 + h * 512
                        nc.tensor.matmul(e2_ps[:, h * 512:(h + 1) * 512],
                                         st_e_sb[:, r0:r0 + 128],
                                         mv_e_sb[:, lo:lo + 512],
                                         start=True, stop=True)
                    sl = slice(j * 1024, (j + 1) * 1024)
                    t1c = kpool.tile([128, 1024], F32, tag="t1c")
                    _i = nc.scalar.activation(t1c[:], e2_ps[:], AF.Square,
                                              bias=1.0)
                    if acts["first_alpha"] is None:
                        acts["first_alpha"] = _i
                    s1c = kpool.tile([128, 1024], F32, tag="s1c")
                    nc.scalar.activation(s1c[:], t1c[:], AF.Sqrt, bias=bias_s1[:])
                    nc.vector._custom_dve(
                        ADD_MAXACC, out=u[:, sl], in0=e2_ps[:], in1=s1c[:],
                        s1=0.0, accum_out=maxu_acc[:, j:j + 1])
                for j in range(n1024):
                    g_ps = psum.tile([128, 1024], F32, tag="g")
                    for h in range(2):
                        lo = j * 1024 + h * 512
                        nc.tensor.matmul(g_ps[:, h * 512:(h + 1) * 512],
                                         st_g_sb[:, r0:r0 + 128],
                                         mv_g_sb[:, lo:lo + 512],
                                         start=True, stop=True)
                    sl = slice(j * 1024, (j + 1) * 1024)
                    gesc = kpool.tile([128, 1024], BF16, tag="gesc")
                    nc.vector._custom_dve(
                        MUL_MAXACC, out=gesc[:], in0=g_ps[:], in1=adj_t[:, sl],
                        s1=-3.0e38, imm2=1.0, accum_out=maxge_acc[:, j:j + 1])
                    gesc2 = kpool.tile([128, 1024], BF16, tag="gesc")
                    nc.vector._custom_dve(
                        MUL_MAXACC, out=gesc2[:], in0=g_ps[:], in1=adj_t[:, sl],
                        s1=-3.0e38, imm2=-1.0, accum_out=negge_acc[:, j:j + 1])
                maxge_v = vpool.tile([128, 1], F32, tag="maxge")
                nc.vector.tensor_reduce(maxge_v[:], maxge_acc[:],
                                        mybir.AxisListType.X, ALU.max)
                minge_v = vpool.tile([128, 1], F32, tag="minge")
                nc.vector.tensor_reduce(minge_v[:], negge_acc[:],
                                        mybir.AxisListType.X, ALU.max)
                maxu_v = vpool.tile([128, 1], F32, tag="maxu")
                nc.vector.tensor_reduce(maxu_v[:], maxu_acc[:],
                                        mybir.AxisListType.X, ALU.max)
                vec[("maxge", it)] = maxge_v
                vec[("minge", it)] = minge_v
                vec[("maxu", it)] = maxu_v

            def beta(it, u, adj_t):
                r0 = it * 128
                ll = w1pool.tile([128, width], F32, tag="ll")
                acts["beta_list"].append(
                    nc.scalar.activation(ll[:], u[:], AF.Ln, bias=1.0))
                sq = w1pool.tile([128, width], F32, tag="sq")
                acts["beta_list"].append(
                    nc.scalar.activation(sq[:], ll[:], AF.Square))
                nc.sync.dma_start(sq_out[r0:r0 + 128, :], sq[:])

                simi = w1pool.tile([128, width], F32, tag="simi")
                sumsimi_v = vpool.tile([128, 1], F32, tag="sumsimi")
                acts["beta_list"].append(
                    nc.scalar.activation(simi[:], sq[:], AF.Exp, scale=-1.0,
                                         accum_out=sumsimi_v[:]))

                scratch = bpool.tile([128, width], BF16, tag="b16")
                possimi_v = vpool.tile([128, 1], F32, tag="possimi")
                nc.vector.scalar_tensor_tensor(
                    scratch[:], simi[:], 0.0, adj_t[:],
                    op0=ALU.add, op1=ALU.mult, accum_out=possimi_v[:])

                negsum_v = vpool.tile([128, 1], F32, tag="negsum")
                nc.vector.tensor_tensor(negsum_v[:], sumsimi_v[:], possimi_v[:],
                                        ALU.subtract)
                m_v = vpool.tile([128, 1], F32, tag="m")
                acts["beta_list"].append(
                    nc.scalar.activation(m_v[:], negsum_v[:], AF.Ln))

                scratch2 = bpool.tile([128, width], BF16, tag="b16")
                lossrow_v = vpool.tile([128, 1], F32, tag="lossrow")
                nc.vector.scalar_tensor_tensor(
                    scratch2[:], sq[:], m_v[:], adj_t[:],
                    op0=ALU.add, op1=ALU.mult, accum_out=lossrow_v[:])

                nc.sync.dma_start(loss_out[it:it + 1, :].rearrange("a b -> b a"),
                                  lossrow_v[:])
                nc.sync.dma_start(maxsq_out[it:it + 1, :].rearrange("a b -> b a"),
                                  vec[("maxu", it)][:])
                nc.sync.dma_start(maxge_out[it:it + 1, :].rearrange("a b -> b a"),
                                  vec[("maxge", it)][:])
                nc.sync.dma_start(minge_out[it:it + 1, :].rearrange("a b -> b a"),
                                  vec[("minge", it)][:])
                nc.sync.dma_start(negsum_out[it:it + 1, :].rearrange("a b -> b a"),
                                  negsum_v[:])

            for p in range(0, rt, 2):
                acts["first_alpha"] = None
                prev_list = acts["beta_list"]
                acts["beta_list"] = []
                tiles = []
                for it in (p, p + 1):
                    if it >= rt:
                        continue
                    adj_t = apool.tile([128, width], I32, tag="adj")
                    nc.sync.dma_start(adj_t[:], adj_b[it * 128:it * 128 + 128, :])
                    u = upool.tile([128, width], F32, tag="u")
                    alpha(it, u, adj_t)
                    tiles.append((it, u, adj_t))
                del prev_list  # phase-ordering deps hurt in the cost model
                for it, u, adj_t in tiles:
                    beta(it, u, adj_t)
    nc.compile()
    return nc


def _prep_tables(x):
    """Feature tables (float64 host math) + bf16x3 splits."""
    xf = x.astype(np.float64)
    nx = (xf * xf).sum(axis=1)
    px = 1.0 - nx
    a = np.concatenate([xf.T / px[None, :],
                        (nx / px)[None, :],
                        (1.0 / px)[None, :]], axis=0)          # [18, N]
    b = np.concatenate([-4.0 * xf.T / px[None, :],
                        (2.0 / px)[None, :],
                        (2.0 * nx / px)[None, :]], axis=0)     # [18, N]
    ah, al = _bf16_split3(a)
    bh, bl = _bf16_split3(b)
    st_e = np.concatenate([ah, ah, al], axis=0)                # [54, N]
    mv_e = np.concatenate([bh, bl, bh], axis=0)                # [54, N]
    g = xf.T                                                   # [16, N]
    gh, gl = _bf16_split3(g)
    st_g = np.concatenate([gh, gh, gl], axis=0)                # [48, N]
    mv_g = np.concatenate([gh, gl, gh], axis=0)                # [48, N]
    return st_e, mv_e, st_g, mv_g


def _run(x, adj, trace=False):
    if "nc" not in _CACHE:
        _CACHE["nc"] = build_program()
    nc = _CACHE["nc"]
    st_e, mv_e, st_g, mv_g = _prep_tables(x)
    in_maps = []
    for c in range(NCORES):
        r0 = c * R
        in_maps.append({
            "st_e": np.ascontiguousarray(st_e[:, r0:r0 + R]),
            "mv_e": np.ascontiguousarray(mv_e),
            "st_g": np.ascontiguousarray(st_g[:, r0:r0 + R]),
            "mv_g": np.ascontiguousarray(mv_g),
            "adj_b": np.ascontiguousarray(adj[r0:r0 + R, :]),
        })
    res = run_bass_kernel_spmd(nc, in_maps, core_ids=list(range(NCORES)),
                               trace=trace)
    return res


def kernel(x, adj):
    x = np.asarray(x, dtype=np.float32)
    adj = np.asarray(adj, dtype=np.int32)
    res = _run(x, adj)

    sq = np.concatenate([res.results[c]["sq_out"] for c in range(NCORES)], axis=0)
    lossrow = np.concatenate(
        [res.results[c]["loss_out"].reshape(-1) for c in range(NCORES)])
    maxsq = np.concatenate(
        [res.results[c]["maxsq_out"].reshape(-1) for c in range(NCORES)])
    maxge = np.concatenate(
        [res.results[c]["maxge_out"].reshape(-1) for c in range(NCORES)])
    minge = np.concatenate(
        [res.results[c]["minge_out"].reshape(-1) for c in range(NCORES)])

    loss = np.float32(lossrow.astype(np.float64).sum())
    dist_max = np.float32(np.log1p(np.float64(maxsq.max())) ** 2)
    max_inner = np.float32(maxge.max())
    min_inner = np.float32(-minge.max())
    return (x, sq, loss, dist_max, max_inner, min_inner)
